# revision 4
# baseline (speedup 1.0000x reference)
"""MultiHeadEMA Trainium2 Bass kernel.

Reference computation (B=4, S=8192, D=1024, N=2):
    out = silu(conv_causal(x, k) + x * omega)
    k[d, l] = sum_n c[d, n] * q[d, n]^l
    q = 1 - sigmoid(delta) * sigmoid(alpha)
    c = sigmoid(delta) * beta * gamma * sqrt(1/N)

The length-S causal conv with a sum-of-2-exponentials kernel is a pair of
first-order linear recurrences (EMA scans):
    h_n[t] = q_n * h_n[t-1] + x[t]
    y[t]   = c_1 h_1[t] + c_2 h_2[t]
    out[t] = silu(y[t] + omega * x[t])

Sharding: D=1024 split across 8 cores (128 channels each).  Each core works
in [channel-partition, time-free] layout; the scans run on the Vector engine
via TensorTensorScanArith, one recurrence per partition.  The host transposes
x to [B, D, S] while slicing the per-core shards and transposes the per-core
results back while gathering (part of the shard/unshard contract).
"""

import math

import numpy as np

import concourse.bass as bass
import concourse.mybir as mybir
import concourse.tile as tile
from concourse import bacc
from concourse.bass_utils import run_bass_kernel_spmd

B = 4
S = 8192
D = 1024
N_CORES = 8
D_LOC = D // N_CORES  # 128 channels per core
SCALE = math.sqrt(1.0 / 2.0)

F32 = mybir.dt.float32


def build_nc(b=B, d_loc=D_LOC, s=S, t_chunk=2048, act="Silu"):
    """Build the per-core Bass module (SPMD: same NEFF on all cores).

    Inputs (per core):
      x  [b, d_loc, s] f32 — time-major-last shard of the input
      pp [d_loc, 8]    f32 — packed params: q1 q2 c1 c2 w (cols 0-4)
    Output:
      o  [b, d_loc, s] f32
    """
    assert s % t_chunk == 0
    n_chunks = s // t_chunk

    nc = bacc.Bacc(
        "TRN2",
        target_bir_lowering=False,
        debug=False,
        enable_asserts=False,
        num_devices=N_CORES,
    )

    x_d = nc.dram_tensor("x", [b, d_loc, s], F32, kind="ExternalInput").ap()
    pp_d = nc.dram_tensor("pp", [d_loc, 8], F32, kind="ExternalInput").ap()
    o_d = nc.dram_tensor("o", [b, d_loc, s], F32, kind="ExternalOutput").ap()

    with tile.TileContext(nc) as tc:
        with (
            tc.tile_pool(name="pp", bufs=1) as pp_pool,
            tc.tile_pool(name="x", bufs=3) as x_pool,
            tc.tile_pool(name="h", bufs=3) as h_pool,
            tc.tile_pool(name="tmp", bufs=3) as tmp_pool,
        ):
            pp = pp_pool.tile([d_loc, 8], F32, tag="pp")
            nc.sync.dma_start(out=pp[:], in_=pp_d[:])
            q1 = pp[:, 0:1]
            q2 = pp[:, 1:2]
            c1 = pp[:, 2:3]
            c2 = pp[:, 3:4]
            w = pp[:, 4:5]
            q1_b = q1.broadcast_to([d_loc, t_chunk])
            q2_b = q2.broadcast_to([d_loc, t_chunk])

            mult = mybir.AluOpType.mult
            add = mybir.AluOpType.add

            h1_prev = None
            h2_prev = None
            for bi in range(b):
                for j in range(n_chunks):
                    t0 = j * t_chunk
                    xt = x_pool.tile([d_loc, t_chunk], F32, tag="x")
                    nc.sync.dma_start(out=xt[:], in_=x_d[bi, :, t0 : t0 + t_chunk])

                    # residual pre-scale on the Scalar engine: t1 = w * x
                    t1 = tmp_pool.tile([d_loc, t_chunk], F32, tag="t1")
                    nc.scalar.activation(
                        t1[:], xt[:], mybir.ActivationFunctionType.Copy, scale=w
                    )

                    # EMA scans on the Vector engine (chained across chunks)
                    i1 = 0.0 if j == 0 else h1_prev[:, t_chunk - 1 : t_chunk]
                    i2 = 0.0 if j == 0 else h2_prev[:, t_chunk - 1 : t_chunk]
                    h1 = h_pool.tile([d_loc, t_chunk], F32, tag="h1")
                    h2 = h_pool.tile([d_loc, t_chunk], F32, tag="h2")
                    nc.vector.tensor_tensor_scan(h1[:], q1_b, xt[:], i1, mult, add)
                    nc.vector.tensor_tensor_scan(h2[:], q2_b, xt[:], i2, mult, add)
                    h1_prev, h2_prev = h1, h2

                    # u = c1*h1 + t1   (Vector engine, fused)
                    u = tmp_pool.tile([d_loc, t_chunk], F32, tag="u")
                    nc.vector.scalar_tensor_tensor(u[:], h1[:], c1, t1[:], mult, add)
                    # h2c = c2*h2 ; r = u + h2c   (GpSimd engine — the fused
                    # scalar_tensor_tensor opcode is not supported on Pool)
                    h2c = tmp_pool.tile([d_loc, t_chunk], F32, tag="h2c")
                    nc.gpsimd.tensor_scalar(h2c[:], h2[:], c2, None, mult)
                    r = tmp_pool.tile([d_loc, t_chunk], F32, tag="r")
                    nc.gpsimd.tensor_tensor(out=r[:], in0=u[:], in1=h2c[:], op=add)

                    # out = silu(r)    (Scalar engine)
                    ot = tmp_pool.tile([d_loc, t_chunk], F32, tag="ot")
                    nc.scalar.activation(
                        ot[:], r[:], getattr(mybir.ActivationFunctionType, act)
                    )
                    nc.sync.dma_start(out=o_d[bi, :, t0 : t0 + t_chunk], in_=ot[:])

    nc.compile()
    return nc


def _host_params(delta, alpha, beta, gamma, omega):
    """Compute per-channel scan params on the host (O(D*N) work)."""
    p = 1.0 / (1.0 + np.exp(-delta[:, :, 0].astype(np.float64)))  # [D, N]
    a = 1.0 / (1.0 + np.exp(-alpha[:, :, 0].astype(np.float64)))
    q = 1.0 - p * a                                               # [D, N]
    c = p * beta[:, :, 0].astype(np.float64) * gamma.astype(np.float64) * SCALE
    pp = np.zeros((D, 8), dtype=np.float32)
    pp[:, 0] = q[:, 0]
    pp[:, 1] = q[:, 1]
    pp[:, 2] = c[:, 0]
    pp[:, 3] = c[:, 1]
    pp[:, 4] = omega
    return pp


_NC_CACHE = {}


def kernel(x, delta, alpha, beta, gamma, omega):
    assert x.shape == (B, S, D) and x.dtype == np.float32

    if "nc" not in _NC_CACHE:
        _NC_CACHE["nc"] = build_nc()
    nc = _NC_CACHE["nc"]

    pp = _host_params(delta, alpha, beta, gamma, omega)
    xt = np.ascontiguousarray(x.transpose(0, 2, 1))  # [B, D, S]

    in_maps = []
    for i in range(N_CORES):
        sl = slice(i * D_LOC, (i + 1) * D_LOC)
        in_maps.append(
            {
                "x": np.ascontiguousarray(xt[:, sl, :]),
                "pp": np.ascontiguousarray(pp[sl]),
            }
        )

    res = run_bass_kernel_spmd(nc, in_maps, core_ids=list(range(N_CORES)))

    out = np.empty((B, S, D), dtype=np.float32)
    for i in range(N_CORES):
        sl = slice(i * D_LOC, (i + 1) * D_LOC)
        out[:, :, sl] = res.results[i]["o"].transpose(0, 2, 1)
    return out


# revision 5
# speedup vs baseline: 2.9828x; 2.9828x over previous
"""MultiHeadEMA Trainium2 Bass kernel.

Reference computation (B=4, S=8192, D=1024, N=2):
    out = silu(conv_causal(x, k) + x * omega)
    k[d, l] = sum_n c[d, n] * q[d, n]^l
    q = 1 - sigmoid(delta) * sigmoid(alpha)
    c = sigmoid(delta) * beta * gamma * sqrt(1/N)

The length-S causal conv with a sum-of-2-exponentials kernel is a pair of
first-order linear recurrences (EMA scans):
    h_n[t] = q_n * h_n[t-1] + x[t]
    y[t]   = c_1 h_1[t] + c_2 h_2[t]
    out[t] = silu(y[t] + omega * x[t])

Sharding: D=1024 split across 8 cores (128 channels each).  Each core works
in [channel-partition, time-free] layout; the scans run on the Vector engine
via TensorTensorScanArith, one recurrence per partition.  The host transposes
x to [B, D, S] while slicing the per-core shards and transposes the per-core
results back while gathering (part of the shard/unshard contract).
"""

import math

import numpy as np

import concourse.bass as bass
import concourse.mybir as mybir
import concourse.tile as tile
from concourse import bacc
from concourse.bass_utils import run_bass_kernel_spmd

B = 4
S = 8192
D = 1024
N_CORES = 8
D_LOC = D // N_CORES  # 128 channels per core
SCALE = math.sqrt(1.0 / 2.0)

F32 = mybir.dt.float32


def build_nc(b=B, d_loc=D_LOC, s=S, t_chunk=2048, act="Silu"):
    """Build the per-core Bass module (SPMD: same NEFF on all cores).

    Inputs (per core):
      x  [b, d_loc, s] f32 — time-major-last shard of the input
      pp [d_loc, 8]    f32 — packed params: q1 q2 c1 c2 w (cols 0-4)
    Output:
      o  [b, d_loc, s] f32
    """
    assert s % t_chunk == 0
    n_chunks = s // t_chunk

    nc = bacc.Bacc(
        "TRN2",
        target_bir_lowering=False,
        debug=False,
        enable_asserts=False,
        num_devices=N_CORES,
    )

    x_d = nc.dram_tensor("x", [b, d_loc, s], F32, kind="ExternalInput").ap()
    pp_d = nc.dram_tensor("pp", [d_loc, 8], F32, kind="ExternalInput").ap()
    o_d = nc.dram_tensor("o", [b, d_loc, s], F32, kind="ExternalOutput").ap()

    with tile.TileContext(nc) as tc:
        with (
            tc.tile_pool(name="pp", bufs=1) as pp_pool,
            tc.tile_pool(name="x", bufs=3) as x_pool,
            tc.tile_pool(name="h", bufs=3) as h_pool,
            tc.tile_pool(name="tmp", bufs=3) as tmp_pool,
        ):
            pp = pp_pool.tile([d_loc, 8], F32, tag="pp")
            nc.sync.dma_start(out=pp[:], in_=pp_d[:])
            q1 = pp[:, 0:1]
            q2 = pp[:, 1:2]
            c1 = pp[:, 2:3]
            c2 = pp[:, 3:4]
            w = pp[:, 4:5]
            q1_b = q1.broadcast_to([d_loc, t_chunk])
            q2_b = q2.broadcast_to([d_loc, t_chunk])

            mult = mybir.AluOpType.mult
            add = mybir.AluOpType.add

            h1_prev = None
            h2_prev = None
            for bi in range(b):
                for j in range(n_chunks):
                    t0 = j * t_chunk
                    xt = x_pool.tile([d_loc, t_chunk], F32, tag="x")
                    nc.sync.dma_start(out=xt[:], in_=x_d[bi, :, t0 : t0 + t_chunk])

                    # residual pre-scale on the Scalar engine: t1 = w * x
                    t1 = tmp_pool.tile([d_loc, t_chunk], F32, tag="t1")
                    nc.scalar.activation(
                        t1[:], xt[:], mybir.ActivationFunctionType.Copy, scale=w
                    )

                    # EMA scans on the Vector engine (chained across chunks)
                    i1 = 0.0 if j == 0 else h1_prev[:, t_chunk - 1 : t_chunk]
                    i2 = 0.0 if j == 0 else h2_prev[:, t_chunk - 1 : t_chunk]
                    h1 = h_pool.tile([d_loc, t_chunk], F32, tag="h1")
                    h2 = h_pool.tile([d_loc, t_chunk], F32, tag="h2")
                    nc.vector.tensor_tensor_scan(h1[:], q1_b, xt[:], i1, mult, add)
                    nc.vector.tensor_tensor_scan(h2[:], q2_b, xt[:], i2, mult, add)
                    h1_prev, h2_prev = h1, h2

                    # u = c1*h1 + t1 ; r = c2*h2 + u   (Vector engine, fused
                    # muladds).  GpSimd is deliberately unused: its streaming
                    # ops hog the shared DVE<->GpSimd SBUF port and stall the
                    # scans (measured 9 c/e scan throughput with GpSimd busy
                    # vs 2.07 c/e without).
                    u = tmp_pool.tile([d_loc, t_chunk], F32, tag="u")
                    nc.vector.scalar_tensor_tensor(u[:], h1[:], c1, t1[:], mult, add)
                    r = tmp_pool.tile([d_loc, t_chunk], F32, tag="r")
                    nc.vector.scalar_tensor_tensor(r[:], h2[:], c2, u[:], mult, add)

                    # out = silu(r)    (Scalar engine)
                    ot = tmp_pool.tile([d_loc, t_chunk], F32, tag="ot")
                    nc.scalar.activation(
                        ot[:], r[:], getattr(mybir.ActivationFunctionType, act)
                    )
                    nc.sync.dma_start(out=o_d[bi, :, t0 : t0 + t_chunk], in_=ot[:])

    nc.compile()
    return nc


def _host_params(delta, alpha, beta, gamma, omega):
    """Compute per-channel scan params on the host (O(D*N) work)."""
    p = 1.0 / (1.0 + np.exp(-delta[:, :, 0].astype(np.float64)))  # [D, N]
    a = 1.0 / (1.0 + np.exp(-alpha[:, :, 0].astype(np.float64)))
    q = 1.0 - p * a                                               # [D, N]
    c = p * beta[:, :, 0].astype(np.float64) * gamma.astype(np.float64) * SCALE
    pp = np.zeros((D, 8), dtype=np.float32)
    pp[:, 0] = q[:, 0]
    pp[:, 1] = q[:, 1]
    pp[:, 2] = c[:, 0]
    pp[:, 3] = c[:, 1]
    pp[:, 4] = omega
    return pp


_NC_CACHE = {}


def kernel(x, delta, alpha, beta, gamma, omega):
    assert x.shape == (B, S, D) and x.dtype == np.float32

    if "nc" not in _NC_CACHE:
        _NC_CACHE["nc"] = build_nc()
    nc = _NC_CACHE["nc"]

    pp = _host_params(delta, alpha, beta, gamma, omega)
    xt = np.ascontiguousarray(x.transpose(0, 2, 1))  # [B, D, S]

    in_maps = []
    for i in range(N_CORES):
        sl = slice(i * D_LOC, (i + 1) * D_LOC)
        in_maps.append(
            {
                "x": np.ascontiguousarray(xt[:, sl, :]),
                "pp": np.ascontiguousarray(pp[sl]),
            }
        )

    res = run_bass_kernel_spmd(nc, in_maps, core_ids=list(range(N_CORES)))

    out = np.empty((B, S, D), dtype=np.float32)
    for i in range(N_CORES):
        sl = slice(i * D_LOC, (i + 1) * D_LOC)
        out[:, :, sl] = res.results[i]["o"].transpose(0, 2, 1)
    return out
